# revision 1
# baseline (speedup 1.0000x reference)
"""Trainium2 Bass kernel for nn_CustomAttn: qkv proj + flat-axis qk-RMSnorm +
RoPE + causal attention + out proj on 8 NeuronCores.

Tensor-parallel over heads across all 8 cores (2 heads/core); both batches are
processed on every core as extra token rows (token axis is batch-major,
4096 = 2*2048).  This puts every shipped byte on exactly one core — the
dominant cost of this harness is host->device input transfer per call, so
inputs are sharded to the minimum:

  - x is shipped contraction-sharded ([256 hid rows, 4096 tok] bf16 per core)
    and AllGather'd on device to the full [2048, 4096].
  - w_in / w_out shards are exactly this core's head slice (no DP duplication).
  - RoPE cos/sin tables are generated on device (iota + Sin activation) from a
    512-byte signed inv_freq column.
  - output is bf16 [512, 2048] per core (1/8 of the full output), assembled
    and upcast on host.

Collectives (all groups = all 8 cores): AllGather(x), AllReduce(qk-norm sum of
squares — the norm spans all 16 heads), and a per-(batch, 512-token-tile)
ReduceScatter of out-projection partials that fires as soon as that tile's
attention + projection are done, overlapping comm with the next tile's compute.
Core c ends with rows p*64..(p+1)*64 of piece p = 4*b+j -> host maps to
batch b, tokens j*512 + c*64 .. j*512 + (c+1)*64.
"""

import sys

for p in ("/opt/trn_rl_repo",):
    if p not in sys.path:
        sys.path.insert(0, p)

import numpy as np
import ml_dtypes
from contextlib import ExitStack

import concourse.bass as bass
import concourse.bacc as bacc
from concourse.tile import TileContext
from concourse import mybir
from concourse.bass_utils import run_bass_kernel_spmd

BF16 = mybir.dt.bfloat16
F32 = mybir.dt.float32
FP8 = mybir.dt.float8e4
NPBF16 = ml_dtypes.bfloat16
NPFP8 = ml_dtypes.float8_e4m3
WSCALE = 16.0                  # fp8 weight pre-scale (folded back out on chip)

B, S, HID = 2, 2048, 2048
NH, HD = 16, 128
EPS = 1e-5
ROPE_BASE = 10000.0

NCORES = 8
NHL = NH // NCORES         # 2 local heads
DL = NHL * HD              # 256 local q/k/v dims
KT = HID // 128            # 16 contraction chunks
NT = S // 512              # 4 token tiles of 512 per batch
TOK = B * S                # 4096 batch-major tokens
NTT = B * NT               # 8 token tiles overall
GROUPS = [[0, 1, 2, 3, 4, 5, 6, 7]]
SCALE = 1.0 / float(np.sqrt(HD))

LAST_EXEC_NS = None
_CACHED_NC = None


def build_nc():
    nc = bacc.Bacc(num_devices=NCORES)

    # blob rows 0:256 = xTs [256,4096]; 256:640 = w_inT [2048,768] flat;
    # 640:768 = w_outT [256,2048] flat (one array -> one host->device ship)
    blob = nc.declare_dram_parameter("blob", [768, TOK], BF16, isOutput=False)
    # packed aux: cols [qn(2)|kn(2)|CA(32)|SA(32)|CB(64)|SB(64)|mask(128)]
    # rope angle tables use pos = 64*hi + lo; SA/SB sign-folded (rows 0:64
    # negated) so si comes out [-sin; sin]
    aux = nc.declare_dram_parameter("aux", [128, 324], F32, isOutput=False)
    out = nc.declare_dram_parameter("out", [512, HID], BF16, isOutput=True)

    NCH = 4                    # x AllGather chunks (pipeline comm vs phase 1)
    CW = TOK // NCH
    xag_in = [nc.dram_tensor(f"xag{c}", [DL, CW], BF16) for c in range(NCH)]
    x_full = [nc.dram_tensor(f"xf{c}", [HID, CW], BF16, addr_space="Shared")
              for c in range(NCH)]
    cc_in = nc.dram_tensor("cc_in", [4, S], F32)
    cc_out = nc.dram_tensor("cc_out", [4, S], F32, addr_space="Shared")
    op_buf = nc.dram_tensor("op_buf", [NTT, 512, HID], BF16)
    rs_out = nc.dram_tensor("rs_out", [NTT, 64, HID], BF16)

    with TileContext(nc) as tc, ExitStack() as ctx:
        consts = ctx.enter_context(tc.tile_pool(name="consts", bufs=1))
        weights = ctx.enter_context(tc.tile_pool(name="weights", bufs=1))
        persist = ctx.enter_context(tc.tile_pool(name="persist", bufs=1))
        xcopy = ctx.enter_context(tc.tile_pool(name="xcopy", bufs=2))
        xpool = ctx.enter_context(tc.tile_pool(name="xpool", bufs=2))
        sqp = ctx.enter_context(tc.tile_pool(name="sqp", bufs=2))
        mmp = ctx.enter_context(tc.tile_pool(name="mmp", bufs=3, space="PSUM"))
        accp = ctx.enter_context(tc.tile_pool(name="accp", bufs=2, space="PSUM"))
        smallp = ctx.enter_context(tc.tile_pool(name="smallp", bufs=2, space="PSUM"))
        rqp = ctx.enter_context(tc.tile_pool(name="rqp", bufs=2))
        attnp = ctx.enter_context(tc.tile_pool(name="attnp", bufs=2))
        expp = ctx.enter_context(tc.tile_pool(name="expp", bufs=4))
        wop = ctx.enter_context(tc.tile_pool(name="wop", bufs=2))
        outp = ctx.enter_context(tc.tile_pool(name="outp", bufs=2))

        # --- constants ---
        ones_col_b = consts.tile([128, 1], BF16)       # lhsT for partition-sum
        nc.vector.memset(ones_col_b, 1.0)
        ones_row = consts.tile([1, 128], F32)          # lhsT for partition bcast
        nc.vector.memset(ones_row, 1.0)
        ones_row_b = consts.tile([1, 128], BF16)
        nc.vector.memset(ones_row_b, 1.0)
        aux_t = consts.tile([128, 324], F32)
        nc.sync.dma_start(out=aux_t, in_=aux[:, :])
        qn_t = aux_t[:, 0:NHL]
        kn_t = aux_t[:, NHL:2 * NHL]
        mask_t = consts.tile([128, 128], BF16)
        nc.vector.tensor_copy(mask_t, aux_t[:, 196:324])
        zeros_b = consts.tile([128, 1], F32)           # explicit bias for Exp
        nc.vector.memset(zeros_b, 0.0)

        # --- stage A: stage x shard into internal DRAM, chunked AllGather
        # (4 chunks so phase 1 can start after the first ~quarter arrives) ---
        xcs = []
        for i in range(DL // 128):
            xc = xcopy.tile([128, TOK], BF16, tag="xc", name=f"xc{i}")
            nc.sync.dma_start(out=xc, in_=blob[i * 128:(i + 1) * 128, :])
            xcs.append(xc)
        for c in range(NCH):
            for i in range(DL // 128):
                nc.sync.dma_start(
                    out=xag_in[c][i * 128:(i + 1) * 128, :],
                    in_=xcs[i][:, c * CW:(c + 1) * CW])
            nc.gpsimd.collective_compute(
                "AllGather", mybir.AluOpType.bypass, replica_groups=GROUPS,
                ins=[xag_in[c][:, :]], outs=[x_full[c][:, :]])

        # --- stage B (overlaps the AllGather): weights + raw rope tables ---
        w_tiles = []
        for k in range(KT):
            wt = weights.tile([128, 3 * DL], BF16, tag=f"w{k}", name=f"w{k}")
            nc.sync.dma_start(out=wt, in_=bass.AP(
                blob, 256 * TOK + k * 128 * (3 * DL),
                [[3 * DL, 128], [1, 3 * DL]]))
            w_tiles.append(wt)
        w_out_tiles = []
        for h in range(NHL):
            wt = persist.tile([128, HID], BF16, tag=f"wo{h}", name=f"wot{h}")
            nc.sync.dma_start(out=wt, in_=bass.AP(
                blob, 640 * TOK + h * 128 * HID, [[HID, 128], [1, HID]]))
            w_out_tiles.append(wt)

        # rope tables via angle addition (HW Sin is range-limited):
        # pos = 64*hi + lo; cos(A+B) = CA*CB - SA*SB, sin(A+B) = SA*CB + CA*SB
        # SA/SB are sign-folded on rows 0:64, so si_raw = [-sin; sin] and
        # co_raw = cos on both halves.
        CA, SA = aux_t[:, 4:36], aux_t[:, 36:68]
        CB, SB = aux_t[:, 68:132], aux_t[:, 132:196]
        si_raw = persist.tile([128, S], BF16, tag="sraw", name="si_raw")
        co_raw = persist.tile([128, S], BF16, tag="craw", name="co_raw")
        for g in range(S // 64):
            gsl = slice(g * 64, (g + 1) * 64)
            ca_g, sa_g = CA[:, g:g + 1], SA[:, g:g + 1]
            t1 = sqp.tile([128, 64], F32, tag="rt1", name="rt1", bufs=2)
            t2 = sqp.tile([128, 64], F32, tag="rt2", name="rt2", bufs=2)
            nc.vector.tensor_scalar_mul(t1, CB, ca_g)
            nc.vector.tensor_scalar_mul(t2, SB, sa_g)
            nc.vector.tensor_sub(co_raw[:, gsl], t1, t2)
            t3 = sqp.tile([128, 64], F32, tag="rt1", name="rt3", bufs=2)
            t4 = sqp.tile([128, 64], F32, tag="rt2", name="rt4", bufs=2)
            nc.vector.tensor_scalar_mul(t3, CB, sa_g)
            nc.vector.tensor_scalar_mul(t4, SB, ca_g)
            nc.vector.tensor_add(si_raw[:, gsl], t3, t4)

        # q/k staging (rope applied in place later): tiles (t*2+h) for
        # t in {q,k}, h in {0,1}: [128 dims, 4096 batch-major tokens] bf16
        qk_tiles = [persist.tile([128, TOK], BF16, tag=f"qk{m}", name=f"qk{m}")
                    for m in range(4)]
        # v in token-major layout: [128 tokens, 256 vdims] per token block
        v_tiles = [persist.tile([128, DL], BF16, tag=f"v{tb}", name=f"v{tb}")
                   for tb in range(TOK // 128)]

        # ---------- phase 1: q/k projection + sum-of-squares ----------
        saved_xt = {}
        for n in range(NTT):
            b, jj = n // NT, n % NT
            ch, co = n // (NCH // 2), (n % (NCH // 2)) * 512
            xt = []
            for k in range(KT):
                t = xpool.tile([128, 512], BF16, tag=f"x{k}", name=f"x{k}")
                nc.sync.dma_start(
                    out=t, in_=x_full[ch][k * 128:(k + 1) * 128,
                                          co:co + 512])
                xt.append(t)

            for ti in range(2):                      # q then k heads
                ssq_ps = smallp.tile([1, 512], F32, tag="ssq")
                for hi in range(NHL):
                    col = ti * DL + hi * 128
                    pq = mmp.tile([128, 512], F32, tag="mm")
                    for k in range(KT):
                        nc.tensor.matmul(
                            pq, w_tiles[k][:, col:col + 128], xt[k],
                            start=(k == 0), stop=(k == KT - 1))
                    sq = sqp.tile([128, 512], BF16, tag="sq")
                    nc.scalar.square(sq, pq)
                    nc.tensor.matmul(ssq_ps, ones_col_b, sq,
                                     start=(hi == 0), stop=(hi == NHL - 1))
                    ncol = (qn_t if ti == 0 else kn_t)[:, hi:hi + 1]
                    nc.scalar.mul(
                        qk_tiles[ti * 2 + hi][:, n * 512:(n + 1) * 512],
                        pq, ncol)
                ssq_s = sqp.tile([1, 512], F32, tag="invd", name="ssq_s",
                                 bufs=2)
                nc.scalar.copy(ssq_s, ssq_ps)
                nc.sync.dma_start(
                    out=cc_in[2 * b + ti:2 * b + ti + 1,
                              jj * 512:(jj + 1) * 512], in_=ssq_s)

            if n < NTT - 2:                          # v-proj inline
                for tbl in range(4):
                    tb = n * 4 + tbl
                    pv = mmp.tile([128, 512], F32, tag="mm")
                    for k in range(KT):
                        nc.tensor.matmul(
                            pv[:, 0:DL], xt[k][:, tbl * 128:(tbl + 1) * 128],
                            w_tiles[k][:, 2 * DL:3 * DL],
                            start=(k == 0), stop=(k == KT - 1))
                    nc.vector.tensor_copy(v_tiles[tb], pv[:, 0:DL])
            else:                                    # keep x for post-AR v-proj
                saved_xt[n] = xt

        # ---------- phase 2: allreduce sumsq; v-proj of the last two tiles
        # runs under it using the still-resident x tiles (xpool bufs=2) ------
        nc.gpsimd.collective_compute(
            "AllReduce", mybir.AluOpType.add, replica_groups=GROUPS,
            ins=[cc_in[:, :]], outs=[cc_out[:, :]])

        for n in (NTT - 2, NTT - 1):
            xt = saved_xt[n]
            for tbl in range(4):
                tb = n * 4 + tbl
                pv = mmp.tile([128, 512], F32, tag="mm")
                for k in range(KT):
                    nc.tensor.matmul(
                        pv[:, 0:DL], xt[k][:, tbl * 128:(tbl + 1) * 128],
                        w_tiles[k][:, 2 * DL:3 * DL],
                        start=(k == 0), stop=(k == KT - 1))
                nc.vector.tensor_copy(v_tiles[tb], pv[:, 0:DL])

        # ---------- phase 3: inv_rms + scaled rope tables + rope ----------
        # rows of cc: r = 2*b + t  (t: 0=q, 1=k) — one packed [4, S] tile
        tot = persist.tile([4, S], F32, tag="tot", name="tot")
        nc.sync.dma_start(out=tot, in_=cc_out[:, :])
        eps_b4 = consts.tile([4, 1], F32)
        nc.vector.memset(eps_b4, EPS)
        nc.scalar.activation(tot, tot, mybir.ActivationFunctionType.Sqrt,
                             bias=eps_b4, scale=1.0 / (NH * HD))
        nc.vector.reciprocal(tot, tot)
        inv_t = [tot[r:r + 1, :] for r in range(4)]

        ci = [persist.tile([128, S], BF16, tag=f"ci{r}", name=f"ci{r}")
              for r in range(4)]
        si = [persist.tile([128, S], BF16, tag=f"si{r}", name=f"si{r}")
              for r in range(4)]
        for r in range(4):
            for j in range(NT):
                sl = slice(j * 512, (j + 1) * 512)
                # matmul rhs must start at partition 0 — stage the row chunk
                stg = sqp.tile([1, 512], F32, tag="invd", name="stg", bufs=2)
                nc.sync.dma_start(out=stg, in_=inv_t[r][:, sl])
                bc = mmp.tile([128, 512], F32, tag="mm")
                nc.tensor.matmul(bc, ones_row, stg, start=True, stop=True)
                nc.vector.tensor_mul(ci[r][:, sl], co_raw[:, sl], bc)
                nc.vector.tensor_mul(si[r][:, sl], si_raw[:, sl], bc)

        # rope in place: qk' = qk*ci + rot(qk)*si  (rot = half-swap via DMA,
        # si carries the [-sin; sin] signs)
        for n in range(NTT):
            b, jj = n // NT, n % NT
            slt = slice(jj * 512, (jj + 1) * 512)
            for m in range(4):
                t = m // 2
                r = 2 * b + t
                qk = qk_tiles[m]
                sl = slice(n * 512, (n + 1) * 512)
                rq = rqp.tile([128, 512], BF16, tag="rq", name="rq")
                nc.sync.dma_start(out=rq[0:64, :], in_=qk[64:128, sl])
                nc.sync.dma_start(out=rq[64:128, :], in_=qk[0:64, sl])
                nc.vector.tensor_mul(qk[:, sl], qk[:, sl], ci[r][:, slt])
                nc.vector.tensor_mul(rq, rq, si[r][:, slt])
                nc.vector.tensor_add(qk[:, sl], qk[:, sl], rq)

        # ---------- phase 4+5: attention (b,j outer) + out proj + RS --------
        for b in range(B):
            for j in range(NT):
                p = b * NT + j
                attn_j = []
                for h in range(NHL):
                    qt_h, kt_h = qk_tiles[h], qk_tiles[2 + h]
                    pv_ps = accp.tile([128, 512], F32, tag="pv")
                    den_ps = smallp.tile([1, 512], F32, tag="ssq")
                    nb = 4 * j + 4
                    for kb in range(nb):
                        rr = kb - 4 * j
                        q_off = max(rr, 0) * 128
                        w = 512 - q_off
                        s_ps = mmp.tile([128, 512], F32, tag="mm")
                        nc.tensor.matmul(
                            s_ps[:, :w],
                            kt_h[:, b * S + kb * 128:b * S + (kb + 1) * 128],
                            qt_h[:, b * S + j * 512 + q_off:
                                 b * S + (j + 1) * 512],
                            start=True, stop=True)
                        ex = expp.tile([128, 512], BF16, tag="exp")
                        nc.scalar.activation(ex[:, :w], s_ps[:, :w],
                                             mybir.ActivationFunctionType.Exp,
                                             bias=zeros_b, scale=SCALE)
                        if rr >= 0:
                            nc.vector.tensor_mul(ex[:, 0:128], ex[:, 0:128],
                                                 mask_t)
                        nc.tensor.matmul(
                            pv_ps[:, q_off:512],
                            v_tiles[b * 16 + kb][:, h * 128:(h + 1) * 128],
                            ex[:, :w],
                            start=(kb == 0), stop=(kb == nb - 1))
                        nc.tensor.matmul(
                            den_ps[0:1, q_off:512], ones_col_b, ex[:, :w],
                            start=(kb == 0), stop=(kb == nb - 1))
                    inv_d = sqp.tile([1, 512], BF16, tag="invd", bufs=2)
                    with nc.allow_low_precision(
                            reason="softmax denom bcast in bf16"):
                        nc.vector.reciprocal(inv_d, den_ps)
                    bc = mmp.tile([128, 512], F32, tag="mm")
                    nc.tensor.matmul(bc, ones_row_b, inv_d,
                                     start=True, stop=True)
                    bc_sb = sqp.tile([128, 512], F32, tag="bcsb",
                                     name="bc_sb", bufs=1)
                    nc.scalar.copy(bc_sb, bc)
                    at = attnp.tile([128, 512], BF16, tag=f"at{h}",
                                    name=f"at{h}")
                    nc.vector.tensor_mul(at, pv_ps, bc_sb)
                    attn_j.append(at)

                # out projection for this (batch, token tile), then RS it
                for tbl in range(4):
                    for cch in range(4):
                        po = mmp.tile([128, 512], F32, tag="mm")
                        for h in range(NHL):
                            nc.tensor.matmul(
                                po, attn_j[h][:, tbl * 128:(tbl + 1) * 128],
                                w_out_tiles[h][:, cch * 512:(cch + 1) * 512],
                                start=(h == 0), stop=(h == NHL - 1))
                        ws = wop.tile([128, 512], BF16, tag="wo")
                        nc.vector.tensor_copy(ws, po)
                        nc.sync.dma_start(
                            out=op_buf[p, tbl * 128:(tbl + 1) * 128,
                                       cch * 512:(cch + 1) * 512], in_=ws)
                nc.gpsimd.collective_compute(
                    "ReduceScatter", mybir.AluOpType.add,
                    replica_groups=GROUPS,
                    ins=[op_buf[p]], outs=[rs_out[p]])

                # this core's 64-row slice of piece p -> output rows p*64..
                fin = outp.tile([64, HID], BF16, tag="fin")
                nc.sync.dma_start(out=fin, in_=rs_out[p])
                nc.sync.dma_start(out=out[p * 64:(p + 1) * 64, :], in_=fin)

    nc.finalize()
    return nc


def make_in_maps(x, w_in, w_out, q_norm_w, k_norm_w):
    x = np.asarray(x, np.float32)
    w_in = np.asarray(w_in, np.float32)
    w_out = np.asarray(w_out, np.float32)
    q_norm_w = np.asarray(q_norm_w, np.float32)
    k_norm_w = np.asarray(k_norm_w, np.float32)

    # [2048 hid, 4096 tok] batch-major tokens
    xT_full = np.concatenate([x[0].T, x[1].T], axis=1).astype(NPBF16)

    half = HD // 2
    inv_freq = 1.0 / (ROPE_BASE ** (np.arange(half, dtype=np.float32) / half))
    f2 = np.concatenate([inv_freq, inv_freq])            # [128]
    sgn = np.concatenate([-np.ones(half), np.ones(half)])  # sign fold rows 0:64
    hi = np.arange(32, dtype=np.float32) * 64.0
    lo = np.arange(64, dtype=np.float32)
    angA = f2[:, None] * hi[None, :]                     # [128, 32]
    angB = f2[:, None] * lo[None, :]                     # [128, 64]
    ctab = np.concatenate([
        np.cos(angA), sgn[:, None] * np.sin(angA),
        np.cos(angB), sgn[:, None] * np.sin(angB),
    ], axis=1)                                           # [128, 192]
    maskT = (np.arange(128)[:, None] <= np.arange(128)[None, :])

    in_maps = []
    for c in range(NCORES):
        rows = np.concatenate([
            w_in[c * DL:(c + 1) * DL],
            w_in[NH * HD + c * DL:NH * HD + (c + 1) * DL],
            w_in[2 * NH * HD + c * DL:2 * NH * HD + (c + 1) * DL],
        ], axis=0)                                  # [768, HID]
        aux = np.concatenate([
            q_norm_w[c * DL:(c + 1) * DL].reshape(NHL, 128).T,
            k_norm_w[c * DL:(c + 1) * DL].reshape(NHL, 128).T,
            ctab, maskT,
        ], axis=1).astype(np.float32)               # [128, 324]
        w_inT_c = np.ascontiguousarray(rows.T).astype(NPBF16)
        w_outT_c = np.ascontiguousarray(
            w_out[:, c * DL:(c + 1) * DL].T).astype(NPBF16)
        blob = np.concatenate([
            xT_full[c * DL:(c + 1) * DL, :],
            w_inT_c.reshape(-1, TOK),
            w_outT_c.reshape(-1, TOK),
        ], axis=0)
        in_maps.append({
            "blob": np.ascontiguousarray(blob),
            "aux": np.ascontiguousarray(aux),
        })
    return in_maps


def assemble(results):
    """results[c] is [512, HID] bf16: rows p*64..(p+1)*64 are this core's rank
    slice of piece p = 4*b+j (batch b, tokens j*512 + c*64 ..)."""
    outp = np.empty((B, S, HID), np.float32)
    for c in range(NCORES):
        r = np.asarray(results[c], dtype=np.float32)
        for p in range(NTT):
            b, j = p // NT, p % NT
            t0 = j * 512 + c * 64
            outp[b, t0:t0 + 64, :] = r[p * 64:(p + 1) * 64, :]
    return outp


def kernel(x, w_in, w_out, q_norm_w, k_norm_w, trace=False):
    global LAST_EXEC_NS, _CACHED_NC
    if _CACHED_NC is None:
        _CACHED_NC = build_nc()
    nc = _CACHED_NC
    in_maps = make_in_maps(x, w_in, w_out, q_norm_w, k_norm_w)
    res = run_bass_kernel_spmd(nc, in_maps, list(range(NCORES)), trace=trace)
    LAST_EXEC_NS = res.exec_time_ns
    return assemble([res.results[c]["out"] for c in range(NCORES)])



# revision 2
# speedup vs baseline: 9.3911x; 9.3911x over previous
"""Trainium2 Bass kernel v3 for nn_CustomAttn: qkv proj + flat-axis qk-RMSnorm +
RoPE + causal attention + out proj on 8 NeuronCores.

Tensor-parallel over heads across all 8 cores (2 heads/core); both batches are
processed on every core as extra token rows (token axis is batch-major,
4096 = 2*2048).  x / w shards are staged on device before the timed loop, so
x is shipped fully replicated (no AllGather).

v3 structure:
  - phase 1 (per 512-token tile): x load (2 batched DMAs), q/k proj,
    sum-of-squares accumulation, norm-weight scale, AND rope applied
    immediately via a PE permutation matmul (rot(x) = [-x2; x1] as
    lhsT = [[0,I],[-I,0]]).  Rope commutes with the later inv_rms scale.
  - phase 2: AllReduce of sum-of-squares (out 32KB), v-proj of the last two
    tiles runs under it.
  - phase 3: inv_rms = rsqrt(mean) broadcast via ones-row matmul; qk tiles
    scaled in place (the only post-AR elementwise work).
  - phase 4/5 per (batch, tile): 2-head causal attention (exp via Act,
    denominator via ones-column matmul), out-proj into a dedicated PSUM pool,
    drain copies alternating DVE/Act into one [128,2048] staging tile per
    row block (1 DMA each), per-tile ReduceScatter (out 262KB), direct
    DRAM->DRAM copy of this core's 64-row slice to the output.

Core c ends with rows p*64..(p+1)*64 of piece p = 4*b+j -> host maps to
batch b, tokens j*512 + c*64 .. j*512 + (c+1)*64.
"""

import sys

for p in ("/opt/trn_rl_repo",):
    if p not in sys.path:
        sys.path.insert(0, p)

import numpy as np
import ml_dtypes
from contextlib import ExitStack

import concourse.bass as bass
import concourse.bacc as bacc
from concourse.tile import TileContext
from concourse import mybir
from concourse.bass_utils import run_bass_kernel_spmd

BF16 = mybir.dt.bfloat16
F32 = mybir.dt.float32
NPBF16 = ml_dtypes.bfloat16

B, S, HID = 2, 2048, 2048
NH, HD = 16, 128
EPS = 1e-5
ROPE_BASE = 10000.0

NCORES = 8
NHL = NH // NCORES         # 2 local heads
DL = NHL * HD              # 256 local q/k/v dims
KT = HID // 128            # 16 contraction chunks
NT = S // 512              # 4 token tiles of 512 per batch
TOK = B * S                # 4096 batch-major tokens
NTT = B * NT               # 8 token tiles overall
GROUPS = [[0, 1, 2, 3, 4, 5, 6, 7]]
SCALE = 1.0 / float(np.sqrt(HD))

# blob rows: 0:2048 = xT full [2048, 4096]; then w_inT [2048,768] flat;
# then w_outT [256,2048] flat
WIN_OFF = HID * TOK
WOUT_OFF = WIN_OFF + HID * 3 * DL
BLOB_ROWS = HID + (HID * 3 * DL) // TOK + (DL * HID) // TOK

# aux cols: [qn(2)|kn(2)|CA(32)|SA(32)|CB(64)|SB(64)|mask(128)|Pt(128)]
AUXC = 452

LAST_EXEC_NS = None
_CACHED_NC = None


def build_nc():
    nc = bacc.Bacc(num_devices=NCORES)

    blob = nc.declare_dram_parameter("blob", [BLOB_ROWS, TOK], BF16, isOutput=False)
    aux = nc.declare_dram_parameter("aux", [128, AUXC], F32, isOutput=False)
    out = nc.declare_dram_parameter("out", [512, HID], BF16, isOutput=True)

    cc_in = [nc.dram_tensor(f"cc_in{b}", [2, S], F32) for b in range(B)]
    cc_out = [nc.dram_tensor(f"cc_out{b}", [2, S], F32, addr_space="Shared")
              for b in range(B)]
    op_buf = nc.dram_tensor("op_buf", [NTT, 512, HID], BF16)
    rs_out = nc.dram_tensor("rs_out", [NTT, 64, HID], BF16)

    with TileContext(nc) as tc, ExitStack() as ctx:
        consts = ctx.enter_context(tc.tile_pool(name="consts", bufs=1))
        weights = ctx.enter_context(tc.tile_pool(name="weights", bufs=1))
        persist = ctx.enter_context(tc.tile_pool(name="persist", bufs=1))
        xpool = ctx.enter_context(tc.tile_pool(name="xpool", bufs=2))
        sqp = ctx.enter_context(tc.tile_pool(name="sqp", bufs=2))
        qsp = ctx.enter_context(tc.tile_pool(name="qsp", bufs=2))
        rqp = ctx.enter_context(tc.tile_pool(name="rqp", bufs=2))
        mmp = ctx.enter_context(tc.tile_pool(name="mmp", bufs=2, space="PSUM"))
        accp = ctx.enter_context(tc.tile_pool(name="accp", bufs=2, space="PSUM"))
        smallp = ctx.enter_context(tc.tile_pool(name="smallp", bufs=2, space="PSUM"))
        opp = ctx.enter_context(tc.tile_pool(name="opp", bufs=2, space="PSUM"))
        attnp = ctx.enter_context(tc.tile_pool(name="attnp", bufs=2))
        expp = ctx.enter_context(tc.tile_pool(name="expp", bufs=4))
        wop = ctx.enter_context(tc.tile_pool(name="wop", bufs=2))

        # --- first-needed DMAs up front: weight quad 0 + x tile 0, so the
        # first projection matmul can start ~5us in ---
        w_quads = []
        wq = weights.tile([128, 4 * 3 * DL], BF16, tag="wf0", name="w_flat0")
        nc.sync.dma_start(out=wq, in_=bass.AP(
            blob, WIN_OFF, [[3 * DL, 128], [128 * 3 * DL, 4], [1, 3 * DL]]))
        w_quads.append(wq)
        x0_flat = xpool.tile([128, KT * 512], BF16, tag="x", name="x")
        for hseg in range(2):
            nc.sync.dma_start(
                out=x0_flat[:, hseg * 8 * 512:(hseg + 1) * 8 * 512],
                in_=bass.AP(
                    blob, (hseg * 8 * 128) * TOK,
                    [[TOK, 128], [128 * TOK, 8], [1, 512]]))

        # --- constants ---
        ones_col_b = consts.tile([128, 1], BF16)       # lhsT for partition-sum
        nc.vector.memset(ones_col_b, 1.0)
        ones_row = consts.tile([1, 128], F32)          # lhsT for partition bcast
        nc.vector.memset(ones_row, 1.0)
        ones_row_b = consts.tile([1, 128], BF16)
        nc.vector.memset(ones_row_b, 1.0)
        aux_t = consts.tile([128, AUXC], F32)
        nc.sync.dma_start(out=aux_t, in_=aux[:, :])
        qn_t = aux_t[:, 0:NHL]
        kn_t = aux_t[:, NHL:2 * NHL]
        mask_t = consts.tile([128, 128], BF16)
        nc.vector.tensor_copy(mask_t, aux_t[:, 196:324])
        rot_t = consts.tile([128, 128], BF16)          # lhsT of rot(x)=[-x2;x1]
        nc.vector.tensor_copy(rot_t, aux_t[:, 324:452])
        zeros_b = consts.tile([128, 1], F32)           # explicit bias for Exp
        nc.vector.memset(zeros_b, 0.0)

        # --- remaining weights ---
        for q in range(1, 4):
            wq = weights.tile([128, 4 * 3 * DL], BF16, tag=f"wf{q}",
                              name=f"w_flat{q}")
            nc.sync.dma_start(out=wq, in_=bass.AP(
                blob, WIN_OFF + q * 4 * 128 * 3 * DL,
                [[3 * DL, 128], [128 * 3 * DL, 4], [1, 3 * DL]]))
            w_quads.append(wq)
        w_tiles = [w_quads[k // 4][:, (k % 4) * 3 * DL:(k % 4 + 1) * 3 * DL]
                   for k in range(KT)]
        w_out_tiles = []
        for h in range(NHL):
            wt = persist.tile([128, HID], BF16, tag=f"wo{h}", name=f"wot{h}")
            nc.sync.dma_start(out=wt, in_=bass.AP(
                blob, WOUT_OFF + h * 128 * HID, [[HID, 128], [1, HID]]))
            w_out_tiles.append(wt)

        # rope tables via angle addition (HW Sin is range-limited):
        # pos = 64*hi + lo; cos(A+B) = CA*CB - SA*SB, sin(A+B) = SA*CB + CA*SB
        CA, SA = aux_t[:, 4:36], aux_t[:, 36:68]
        CB, SB = aux_t[:, 68:132], aux_t[:, 132:196]
        si_raw = persist.tile([128, S], BF16, tag="sraw", name="si_raw")
        co_raw = persist.tile([128, S], BF16, tag="craw", name="co_raw")
        for g in range(S // 64):
            gsl = slice(g * 64, (g + 1) * 64)
            ca_g, sa_g = CA[:, g:g + 1], SA[:, g:g + 1]
            t1 = sqp.tile([128, 64], F32, tag="rt1", name="rt1", bufs=2)
            t2 = sqp.tile([128, 64], F32, tag="rt2", name="rt2", bufs=2)
            nc.vector.tensor_scalar_mul(t1, CB, ca_g)
            nc.vector.tensor_scalar_mul(t2, SB, sa_g)
            nc.vector.tensor_sub(co_raw[:, gsl], t1, t2)
            t3 = sqp.tile([128, 64], F32, tag="rt1", name="rt3", bufs=2)
            t4 = sqp.tile([128, 64], F32, tag="rt2", name="rt4", bufs=2)
            nc.vector.tensor_scalar_mul(t3, CB, sa_g)
            nc.vector.tensor_scalar_mul(t4, SB, ca_g)
            nc.vector.tensor_add(si_raw[:, gsl], t3, t4)

        # q/k staging: tiles (t*2+h) for t in {q,k}, h in {0,1}:
        # [128 dims, 4096 batch-major tokens] bf16, roped but not yet
        # inv_rms-scaled
        qk_tiles = [persist.tile([128, TOK], BF16, tag=f"qk{m}", name=f"qk{m}")
                    for m in range(4)]
        # v in token-major layout: [128 tokens, 256 vdims] per token block
        v_tiles = [persist.tile([128, DL], BF16, tag=f"v{tb}", name=f"v{tb}")
                   for tb in range(TOK // 128)]

        # ---------- phase 1: q/k projection + sumsq + rope; per-batch AR
        # fires as soon as that batch's 4 tiles are done, so batch 0's
        # AllReduce latency hides under batch 1's projection ----------
        for b in range(B):
            for jj in range(NT):
                n = b * NT + jj
                slt = slice(jj * 512, (jj + 1) * 512)
                if n == 0:
                    xt_flat = x0_flat
                else:
                    xt_flat = xpool.tile([128, KT * 512], BF16, tag="x",
                                         name="x")
                    for hseg in range(2):
                        nc.sync.dma_start(
                            out=xt_flat[:, hseg * 8 * 512:(hseg + 1) * 8 * 512],
                            in_=bass.AP(
                                blob, (hseg * 8 * 128) * TOK + n * 512,
                                [[TOK, 128], [128 * TOK, 8], [1, 512]]))
                xt = [xt_flat[:, k * 512:(k + 1) * 512] for k in range(KT)]

                for ti in range(2):                  # q then k heads
                    ssq_ps = smallp.tile([1, 512], F32, tag="ssq")
                    for hi in range(NHL):
                        m = ti * 2 + hi
                        col = ti * DL + hi * 128
                        pq = mmp.tile([128, 512], F32, tag="mm")
                        for k in range(KT):
                            nc.tensor.matmul(
                                pq, w_tiles[k][:, col:col + 128], xt[k],
                                start=(k == 0), stop=(k == KT - 1))
                        sq = sqp.tile([128, 512], BF16, tag="sq")
                        nc.scalar.square(sq, pq)
                        nc.tensor.matmul(ssq_ps, ones_col_b, sq,
                                         start=(hi == 0), stop=(hi == NHL - 1))
                        ncol = (qn_t if ti == 0 else kn_t)[:, hi:hi + 1]
                        qs = qsp.tile([128, 512], BF16, tag="qs", name="qs")
                        nc.scalar.mul(qs, pq, ncol)
                        # rope: qk = qs*co + rot(qs)*si
                        rot_ps = accp.tile([128, 512], F32, tag="pv")
                        nc.tensor.matmul(rot_ps, rot_t, qs,
                                         start=True, stop=True)
                        sl = slice(n * 512, (n + 1) * 512)
                        rq = rqp.tile([128, 512], BF16, tag="rq", name="rq")
                        nc.vector.tensor_mul(rq, rot_ps, si_raw[:, slt])
                        nc.vector.tensor_mul(qk_tiles[m][:, sl], qs,
                                             co_raw[:, slt])
                        nc.vector.tensor_add(qk_tiles[m][:, sl],
                                             qk_tiles[m][:, sl], rq)
                    ssq_s = sqp.tile([1, 512], F32, tag="invd", name="ssq_s",
                                     bufs=2)
                    nc.scalar.copy(ssq_s, ssq_ps)
                    nc.sync.dma_start(
                        out=cc_in[b][ti:ti + 1, slt], in_=ssq_s)

                for tbl in range(4):                 # v-proj inline
                    tb = n * 4 + tbl
                    pv = mmp.tile([128, 512], F32, tag="mm")
                    for k in range(KT):
                        nc.tensor.matmul(
                            pv[:, 0:DL], xt[k][:, tbl * 128:(tbl + 1) * 128],
                            w_tiles[k][:, 2 * DL:3 * DL],
                            start=(k == 0), stop=(k == KT - 1))
                    nc.vector.tensor_copy(v_tiles[tb], pv[:, 0:DL])

            nc.gpsimd.collective_compute(
                "AllReduce", mybir.AluOpType.add, replica_groups=GROUPS,
                ins=[cc_in[b][:, :]], outs=[cc_out[b][:, :]])

        # ---------- phase 3+4+5 per batch: inv_rms scale, attention,
        # out proj, RS ----------
        eps_b = consts.tile([1, 1], F32)
        nc.vector.memset(eps_b, EPS)
        for b in range(B):
            inv_rows = []
            for t in range(2):
                tr = persist.tile([1, S], F32, tag=f"inv{2*b+t}",
                                  name=f"inv{2*b+t}")
                nc.sync.dma_start(out=tr, in_=cc_out[b][t:t + 1, :])
                nc.scalar.activation(tr, tr,
                                     mybir.ActivationFunctionType.Sqrt,
                                     bias=eps_b, scale=1.0 / (NH * HD))
                nc.vector.reciprocal(tr, tr)
                inv_rows.append(tr)

            for jj in range(NT):
                n = b * NT + jj
                sl = slice(n * 512, (n + 1) * 512)
                for t in range(2):
                    bc = opp.tile([128, 512], F32, tag="op")
                    nc.tensor.matmul(bc, ones_row,
                                     inv_rows[t][:, jj * 512:(jj + 1) * 512],
                                     start=True, stop=True)
                    for hi in range(NHL):
                        m = t * 2 + hi
                        nc.vector.tensor_mul(qk_tiles[m][:, sl],
                                             qk_tiles[m][:, sl], bc)

            for j in range(NT):
                p = b * NT + j
                attn_j = []
                for h in range(NHL):
                    qt_h, kt_h = qk_tiles[h], qk_tiles[2 + h]
                    pv_ps = accp.tile([128, 512], F32, tag="pv")
                    den_ps = smallp.tile([1, 512], F32, tag="ssq")
                    nb = 4 * j + 4
                    for kb in range(nb):
                        rr = kb - 4 * j
                        q_off = max(rr, 0) * 128
                        w = 512 - q_off
                        s_ps = mmp.tile([128, 512], F32, tag="mm")
                        nc.tensor.matmul(
                            s_ps[:, :w],
                            kt_h[:, b * S + kb * 128:b * S + (kb + 1) * 128],
                            qt_h[:, b * S + j * 512 + q_off:
                                 b * S + (j + 1) * 512],
                            start=True, stop=True)
                        ex = expp.tile([128, 512], BF16, tag="exp")
                        nc.scalar.activation(ex[:, :w], s_ps[:, :w],
                                             mybir.ActivationFunctionType.Exp,
                                             bias=zeros_b, scale=SCALE)
                        if rr >= 0:
                            nc.vector.tensor_mul(ex[:, 0:128], ex[:, 0:128],
                                                 mask_t)
                        nc.tensor.matmul(
                            pv_ps[:, q_off:512],
                            v_tiles[b * 16 + kb][:, h * 128:(h + 1) * 128],
                            ex[:, :w],
                            start=(kb == 0), stop=(kb == nb - 1))
                        nc.tensor.matmul(
                            den_ps[0:1, q_off:512], ones_col_b, ex[:, :w],
                            start=(kb == 0), stop=(kb == nb - 1))
                    inv_d = sqp.tile([1, 512], BF16, tag="invd", bufs=2)
                    with nc.allow_low_precision(
                            reason="softmax denom bcast in bf16"):
                        nc.vector.reciprocal(inv_d, den_ps)
                    bc = opp.tile([128, 512], F32, tag="op")
                    nc.tensor.matmul(bc, ones_row_b, inv_d,
                                     start=True, stop=True)
                    bc_sb = sqp.tile([128, 512], F32, tag="bcsb",
                                     name="bc_sb", bufs=1)
                    nc.scalar.copy(bc_sb, bc)
                    at = attnp.tile([128, 512], BF16, tag=f"at{h}",
                                    name=f"at{h}")
                    nc.vector.tensor_mul(at, pv_ps, bc_sb)
                    attn_j.append(at)

                # out projection for this (batch, token tile), then RS it
                for tbl in range(4):
                    ws = wop.tile([128, HID], BF16, tag="wo")
                    for cch in range(4):
                        po = opp.tile([128, 512], F32, tag="op")
                        for h in range(NHL):
                            nc.tensor.matmul(
                                po, attn_j[h][:, tbl * 128:(tbl + 1) * 128],
                                w_out_tiles[h][:, cch * 512:(cch + 1) * 512],
                                start=(h == 0), stop=(h == NHL - 1))
                        dst = ws[:, cch * 512:(cch + 1) * 512]
                        if cch % 2 == 0:
                            nc.vector.tensor_copy(dst, po)
                        else:
                            nc.scalar.copy(dst, po)
                    nc.sync.dma_start(
                        out=op_buf[p, tbl * 128:(tbl + 1) * 128, :], in_=ws)
                nc.gpsimd.collective_compute(
                    "ReduceScatter", mybir.AluOpType.add,
                    replica_groups=GROUPS,
                    ins=[op_buf[p]], outs=[rs_out[p]])

        # final copies rs_out -> out deferred to the end: issuing them
        # per-tile would park a DMA behind the in-flight RS on the FIFO DMA
        # queue and convoy the next tile's op_buf writes behind it
        for p in range(NTT):
            nc.sync.dma_start(out=out[p * 64:(p + 1) * 64, :], in_=rs_out[p])

    nc.finalize()
    return nc


def make_in_maps(x, w_in, w_out, q_norm_w, k_norm_w):
    x = np.asarray(x, np.float32)
    w_in = np.asarray(w_in, np.float32)
    w_out = np.asarray(w_out, np.float32)
    q_norm_w = np.asarray(q_norm_w, np.float32)
    k_norm_w = np.asarray(k_norm_w, np.float32)

    # [2048 hid, 4096 tok] batch-major tokens
    xT_full = np.concatenate([x[0].T, x[1].T], axis=1).astype(NPBF16)

    half = HD // 2
    inv_freq = 1.0 / (ROPE_BASE ** (np.arange(half, dtype=np.float32) / half))
    f2 = np.concatenate([inv_freq, inv_freq])            # [128]
    hi = np.arange(32, dtype=np.float32) * 64.0
    lo = np.arange(64, dtype=np.float32)
    angA = f2[:, None] * hi[None, :]                     # [128, 32]
    angB = f2[:, None] * lo[None, :]                     # [128, 64]
    ctab = np.concatenate([
        np.cos(angA), np.sin(angA),
        np.cos(angB), np.sin(angB),
    ], axis=1)                                           # [128, 192]
    maskT = (np.arange(128)[:, None] <= np.arange(128)[None, :])
    # lhsT of the rotate-half permutation: rot(x) = [-x2; x1]
    rotT = np.zeros((128, 128), np.float32)
    rotT[0:64, 64:128] = np.eye(64)
    rotT[64:128, 0:64] = -np.eye(64)

    in_maps = []
    for c in range(NCORES):
        rows = np.concatenate([
            w_in[c * DL:(c + 1) * DL],
            w_in[NH * HD + c * DL:NH * HD + (c + 1) * DL],
            w_in[2 * NH * HD + c * DL:2 * NH * HD + (c + 1) * DL],
        ], axis=0)                                  # [768, HID]
        aux = np.concatenate([
            q_norm_w[c * DL:(c + 1) * DL].reshape(NHL, 128).T,
            k_norm_w[c * DL:(c + 1) * DL].reshape(NHL, 128).T,
            ctab, maskT, rotT,
        ], axis=1).astype(np.float32)               # [128, 452]
        w_inT_c = np.ascontiguousarray(rows.T).astype(NPBF16)
        w_outT_c = np.ascontiguousarray(
            w_out[:, c * DL:(c + 1) * DL].T).astype(NPBF16)
        blob = np.concatenate([
            xT_full,
            w_inT_c.reshape(-1, TOK),
            w_outT_c.reshape(-1, TOK),
        ], axis=0)
        in_maps.append({
            "blob": np.ascontiguousarray(blob),
            "aux": np.ascontiguousarray(aux),
        })
    return in_maps


def assemble(results):
    """results[c] is [512, HID] bf16: rows p*64..(p+1)*64 are this core's rank
    slice of piece p = 4*b+j (batch b, tokens j*512 + c*64 ..)."""
    outp = np.empty((B, S, HID), np.float32)
    for c in range(NCORES):
        r = np.asarray(results[c], dtype=np.float32)
        for p in range(NTT):
            b, j = p // NT, p % NT
            t0 = j * 512 + c * 64
            outp[b, t0:t0 + 64, :] = r[p * 64:(p + 1) * 64, :]
    return outp


def kernel(x, w_in, w_out, q_norm_w, k_norm_w, trace=False):
    global LAST_EXEC_NS, _CACHED_NC
    if _CACHED_NC is None:
        _CACHED_NC = build_nc()
    nc = _CACHED_NC
    in_maps = make_in_maps(x, w_in, w_out, q_norm_w, k_norm_w)
    res = run_bass_kernel_spmd(nc, in_maps, list(range(NCORES)), trace=trace)
    LAST_EXEC_NS = res.exec_time_ns
    return assemble([res.results[c]["out"] for c in range(NCORES)])


# revision 4
# speedup vs baseline: 31.2777x; 3.3306x over previous
"""Trainium2 Bass kernel for nn_CustomAttn: qkv proj + flat-axis qk-RMSnorm +
RoPE + causal attention + out proj on 8 NeuronCores.

Tensor-parallel over heads across all 8 cores (2 heads/core); both batches are
processed on every core as extra token rows (token axis is batch-major,
4096 = 2*2048).  All inputs (x replicated, per-core w shards) are staged on
device before the timed loop, so no input collectives are needed.

Structure:
  - phase 1 (per 512-token tile): batched x load, q/k proj (16x128
    contraction chunks), sum-of-squares accumulation via ones-column matmul,
    norm-weight scale, and rope applied immediately via a PE permutation
    matmul (rot(x) = [-x2; x1] as lhsT = [[0,I],[-I,0]]); v-proj inline.
    Rope commutes with the later inv_rms scale (q/k norm weights are
    per-dim but rope-pair-symmetric for this module).
  - per-batch AllReduce of sum-of-squares (out 16KB) fires as soon as that
    batch's 4 tiles are done; batch 0's AR latency hides under batch 1's
    projection, batch 1's under batch 0's attention.
  - inv_rms = rsqrt(mean) broadcast via ones-row matmul; qk tiles scaled in
    place (the only post-AR elementwise work).
  - per (batch, tile): 2-head causal attention with scores issued 2 k-blocks
    ahead (PE never waits the Exp drain), softmax denominator via ones-column
    matmul accumulation, out-proj into a dedicated PSUM pool with drain
    copies alternating DVE/Act into one [128,2048] staging tile per row
    block, per-tile ReduceScatter (out 262KB).
  - rs_out -> out copies are deferred to the end: issuing them per-tile
    would park a DMA behind the in-flight RS on the FIFO DMA queue and
    convoy the next tile's op_buf writes behind it.

Core c ends with rows p*64..(p+1)*64 of piece p = 4*b+j -> host maps to
batch b, tokens j*512 + c*64 .. j*512 + (c+1)*64.
"""

import sys

for p in ("/opt/trn_rl_repo",):
    if p not in sys.path:
        sys.path.insert(0, p)

import numpy as np
import ml_dtypes
from contextlib import ExitStack

import concourse.bass as bass
import concourse.bacc as bacc
from concourse.tile import TileContext
from concourse import mybir
from concourse.bass_utils import run_bass_kernel_spmd

BF16 = mybir.dt.bfloat16
F32 = mybir.dt.float32
NPBF16 = ml_dtypes.bfloat16

B, S, HID = 2, 2048, 2048
NH, HD = 16, 128
EPS = 1e-5
ROPE_BASE = 10000.0

NCORES = 8
NHL = NH // NCORES         # 2 local heads
DL = NHL * HD              # 256 local q/k/v dims
KT = HID // 128            # 16 contraction chunks
NT = S // 512              # 4 token tiles of 512 per batch
TOK = B * S                # 4096 batch-major tokens
NTT = B * NT               # 8 token tiles overall
GROUPS = [[0, 1, 2, 3, 4, 5, 6, 7]]
SCALE = 1.0 / float(np.sqrt(HD))

# blob rows: 0:2048 = xT full [2048, 4096]; then w_inT [2048,768] flat;
# then w_outT [256,2048] flat
WIN_OFF = HID * TOK
WOUT_OFF = WIN_OFF + HID * 3 * DL
BLOB_ROWS = HID + (HID * 3 * DL) // TOK + (DL * HID) // TOK

# aux cols: [qn(2)|kn(2)|CA(32)|SA(32)|CB(64)|SB(64)|mask(128)|Pt(128)]
AUXC = 452

LAST_EXEC_NS = None
_CACHED_NC = None


def build_nc():
    nc = bacc.Bacc(num_devices=NCORES)

    blob = nc.declare_dram_parameter("blob", [BLOB_ROWS, TOK], BF16, isOutput=False)
    aux = nc.declare_dram_parameter("aux", [128, AUXC], F32, isOutput=False)
    out = nc.declare_dram_parameter("out", [512, HID], BF16, isOutput=True)

    cc_in = [nc.dram_tensor(f"cc_in{b}", [2, S], F32) for b in range(B)]
    cc_out = [nc.dram_tensor(f"cc_out{b}", [2, S], F32, addr_space="Shared")
              for b in range(B)]
    op_buf = nc.dram_tensor("op_buf", [NTT, 512, HID], BF16)
    rs_out = nc.dram_tensor("rs_out", [NTT, 64, HID], BF16)

    with TileContext(nc) as tc, ExitStack() as ctx:
        consts = ctx.enter_context(tc.tile_pool(name="consts", bufs=1))
        weights = ctx.enter_context(tc.tile_pool(name="weights", bufs=1))
        persist = ctx.enter_context(tc.tile_pool(name="persist", bufs=1))
        xpool = ctx.enter_context(tc.tile_pool(name="xpool", bufs=2))
        sqp = ctx.enter_context(tc.tile_pool(name="sqp", bufs=2))
        qsp = ctx.enter_context(tc.tile_pool(name="qsp", bufs=2))
        rqp = ctx.enter_context(tc.tile_pool(name="rqp", bufs=2))
        mmp = ctx.enter_context(tc.tile_pool(name="mmp", bufs=3, space="PSUM"))
        accp = ctx.enter_context(tc.tile_pool(name="accp", bufs=2, space="PSUM"))
        smallp = ctx.enter_context(tc.tile_pool(name="smallp", bufs=1, space="PSUM"))
        opp = ctx.enter_context(tc.tile_pool(name="opp", bufs=2, space="PSUM"))
        attnp = ctx.enter_context(tc.tile_pool(name="attnp", bufs=2))
        expp = ctx.enter_context(tc.tile_pool(name="expp", bufs=4))
        wop = ctx.enter_context(tc.tile_pool(name="wop", bufs=2))

        # --- first-needed DMAs up front: weight quad 0 + x tile 0, so the
        # first projection matmul can start ~5us in ---
        w_quads = []
        wq = weights.tile([128, 4 * 3 * DL], BF16, tag="wf0", name="w_flat0")
        nc.sync.dma_start(out=wq, in_=bass.AP(
            blob, WIN_OFF, [[3 * DL, 128], [128 * 3 * DL, 4], [1, 3 * DL]]))
        w_quads.append(wq)
        x0_flat = xpool.tile([128, KT * 512], BF16, tag="x", name="x")
        for hseg in range(2):
            nc.sync.dma_start(
                out=x0_flat[:, hseg * 8 * 512:(hseg + 1) * 8 * 512],
                in_=bass.AP(
                    blob, (hseg * 8 * 128) * TOK,
                    [[TOK, 128], [128 * TOK, 8], [1, 512]]))

        # --- constants ---
        ones_col_b = consts.tile([128, 1], BF16)       # lhsT for partition-sum
        nc.vector.memset(ones_col_b, 1.0)
        ones_row = consts.tile([1, 128], F32)          # lhsT for partition bcast
        nc.vector.memset(ones_row, 1.0)
        ones_row_b = consts.tile([1, 128], BF16)
        nc.vector.memset(ones_row_b, 1.0)
        aux_t = consts.tile([128, AUXC], F32)
        nc.sync.dma_start(out=aux_t, in_=aux[:, :])
        qn_t = aux_t[:, 0:NHL]
        kn_t = aux_t[:, NHL:2 * NHL]
        mask_t = consts.tile([128, 128], BF16)
        nc.vector.tensor_copy(mask_t, aux_t[:, 196:324])
        rot_t = consts.tile([128, 128], BF16)          # lhsT of rot(x)=[-x2;x1]
        nc.vector.tensor_copy(rot_t, aux_t[:, 324:452])
        zeros_b = consts.tile([128, 1], F32)           # explicit bias for Exp
        nc.vector.memset(zeros_b, 0.0)

        # --- remaining weights ---
        for q in range(1, 4):
            wq = weights.tile([128, 4 * 3 * DL], BF16, tag=f"wf{q}",
                              name=f"w_flat{q}")
            nc.sync.dma_start(out=wq, in_=bass.AP(
                blob, WIN_OFF + q * 4 * 128 * 3 * DL,
                [[3 * DL, 128], [128 * 3 * DL, 4], [1, 3 * DL]]))
            w_quads.append(wq)
        w_tiles = [w_quads[k // 4][:, (k % 4) * 3 * DL:(k % 4 + 1) * 3 * DL]
                   for k in range(KT)]
        w_out_tiles = []
        for h in range(NHL):
            wt = persist.tile([128, HID], BF16, tag=f"wo{h}", name=f"wot{h}")
            nc.sync.dma_start(out=wt, in_=bass.AP(
                blob, WOUT_OFF + h * 128 * HID, [[HID, 128], [1, HID]]))
            w_out_tiles.append(wt)

        # rope tables via angle addition (HW Sin is range-limited):
        # pos = 64*hi + lo; cos(A+B) = CA*CB - SA*SB, sin(A+B) = SA*CB + CA*SB
        CA, SA = aux_t[:, 4:36], aux_t[:, 36:68]
        CB, SB = aux_t[:, 68:132], aux_t[:, 132:196]
        si_raw = persist.tile([128, S], BF16, tag="sraw", name="si_raw")
        co_raw = persist.tile([128, S], BF16, tag="craw", name="co_raw")
        for g in range(S // 64):
            gsl = slice(g * 64, (g + 1) * 64)
            ca_g, sa_g = CA[:, g:g + 1], SA[:, g:g + 1]
            t1 = sqp.tile([128, 64], F32, tag="rt1", name="rt1", bufs=2)
            t2 = sqp.tile([128, 64], F32, tag="rt2", name="rt2", bufs=2)
            nc.vector.tensor_scalar_mul(t1, CB, ca_g)
            nc.vector.tensor_scalar_mul(t2, SB, sa_g)
            nc.vector.tensor_sub(co_raw[:, gsl], t1, t2)
            t3 = sqp.tile([128, 64], F32, tag="rt1", name="rt3", bufs=2)
            t4 = sqp.tile([128, 64], F32, tag="rt2", name="rt4", bufs=2)
            nc.vector.tensor_scalar_mul(t3, CB, sa_g)
            nc.vector.tensor_scalar_mul(t4, SB, ca_g)
            nc.vector.tensor_add(si_raw[:, gsl], t3, t4)

        # q/k staging: tiles (t*2+h) for t in {q,k}, h in {0,1}:
        # [128 dims, 4096 batch-major tokens] bf16, roped but not yet
        # inv_rms-scaled
        qk_tiles = [persist.tile([128, TOK], BF16, tag=f"qk{m}", name=f"qk{m}")
                    for m in range(4)]
        # v in token-major layout: [128 tokens, 256 vdims] per token block
        v_tiles = [persist.tile([128, DL], BF16, tag=f"v{tb}", name=f"v{tb}")
                   for tb in range(TOK // 128)]

        # ---------- phase 1: q/k projection + sumsq + rope; per-batch AR
        # fires as soon as that batch's 4 tiles are done, so batch 0's
        # AllReduce latency hides under batch 1's projection ----------
        for b in range(B):
            for jj in range(NT):
                n = b * NT + jj
                slt = slice(jj * 512, (jj + 1) * 512)
                if n == 0:
                    xt_flat = x0_flat
                else:
                    xt_flat = xpool.tile([128, KT * 512], BF16, tag="x",
                                         name="x")
                    for hseg in range(2):
                        nc.sync.dma_start(
                            out=xt_flat[:, hseg * 8 * 512:(hseg + 1) * 8 * 512],
                            in_=bass.AP(
                                blob, (hseg * 8 * 128) * TOK + n * 512,
                                [[TOK, 128], [128 * TOK, 8], [1, 512]]))
                xt = [xt_flat[:, k * 512:(k + 1) * 512] for k in range(KT)]

                for ti in range(2):                  # q then k heads
                    ssq_ps = smallp.tile([1, 512], F32, tag="ssq")
                    for hi in range(NHL):
                        m = ti * 2 + hi
                        col = ti * DL + hi * 128
                        pq = mmp.tile([128, 512], F32, tag="mm")
                        for k in range(KT):
                            nc.tensor.matmul(
                                pq, w_tiles[k][:, col:col + 128], xt[k],
                                start=(k == 0), stop=(k == KT - 1))
                        sq = sqp.tile([128, 512], BF16, tag="sq")
                        nc.scalar.square(sq, pq)
                        nc.tensor.matmul(ssq_ps, ones_col_b, sq,
                                         start=(hi == 0), stop=(hi == NHL - 1))
                        ncol = (qn_t if ti == 0 else kn_t)[:, hi:hi + 1]
                        qs = qsp.tile([128, 512], BF16, tag="qs", name="qs")
                        nc.scalar.mul(qs, pq, ncol)
                        # rope: qk = qs*co + rot(qs)*si
                        rot_ps = accp.tile([128, 512], F32, tag="pv")
                        nc.tensor.matmul(rot_ps, rot_t, qs,
                                         start=True, stop=True)
                        sl = slice(n * 512, (n + 1) * 512)
                        rq = rqp.tile([128, 512], BF16, tag="rq", name="rq")
                        nc.vector.tensor_mul(rq, rot_ps, si_raw[:, slt])
                        nc.vector.tensor_mul(qk_tiles[m][:, sl], qs,
                                             co_raw[:, slt])
                        nc.vector.tensor_add(qk_tiles[m][:, sl],
                                             qk_tiles[m][:, sl], rq)
                    ssq_s = sqp.tile([1, 512], F32, tag="invd", name="ssq_s",
                                     bufs=2)
                    nc.scalar.copy(ssq_s, ssq_ps)
                    nc.sync.dma_start(
                        out=cc_in[b][ti:ti + 1, slt], in_=ssq_s)

                for tbl in range(4):                 # v-proj inline
                    tb = n * 4 + tbl
                    pv = mmp.tile([128, 512], F32, tag="mm")
                    for k in range(KT):
                        nc.tensor.matmul(
                            pv[:, 0:DL], xt[k][:, tbl * 128:(tbl + 1) * 128],
                            w_tiles[k][:, 2 * DL:3 * DL],
                            start=(k == 0), stop=(k == KT - 1))
                    nc.vector.tensor_copy(v_tiles[tb], pv[:, 0:DL])

            nc.gpsimd.collective_compute(
                "AllReduce", mybir.AluOpType.add, replica_groups=GROUPS,
                ins=[cc_in[b][:, :]], outs=[cc_out[b][:, :]])

        # ---------- phase 3+4+5 per batch: inv_rms scale, attention,
        # out proj, RS ----------
        eps_b = consts.tile([1, 1], F32)
        nc.vector.memset(eps_b, EPS)
        for b in range(B):
            inv_rows = []
            for t in range(2):
                tr = persist.tile([1, S], F32, tag=f"inv{2*b+t}",
                                  name=f"inv{2*b+t}")
                nc.sync.dma_start(out=tr, in_=cc_out[b][t:t + 1, :])
                nc.scalar.activation(tr, tr,
                                     mybir.ActivationFunctionType.Sqrt,
                                     bias=eps_b, scale=1.0 / (NH * HD))
                nc.vector.reciprocal(tr, tr)
                inv_rows.append(tr)

            for jj in range(NT):
                n = b * NT + jj
                sl = slice(n * 512, (n + 1) * 512)
                for t in range(2):
                    bc = opp.tile([128, 512], F32, tag="op")
                    nc.tensor.matmul(bc, ones_row,
                                     inv_rows[t][:, jj * 512:(jj + 1) * 512],
                                     start=True, stop=True)
                    for hi in range(NHL):
                        m = t * 2 + hi
                        nc.vector.tensor_mul(qk_tiles[m][:, sl],
                                             qk_tiles[m][:, sl], bc)

            for j in range(NT):
                p = b * NT + j
                attn_j = []
                for h in range(NHL):
                    qt_h, kt_h = qk_tiles[h], qk_tiles[2 + h]
                    pv_ps = accp.tile([128, 512], F32, tag="pv")
                    den_ps = smallp.tile([1, 512], F32, tag="ssq")
                    nb = 4 * j + 4

                    def issue_score(kb):
                        q_off = max(kb - 4 * j, 0) * 128
                        w = 512 - q_off
                        s_ps = mmp.tile([128, 512], F32, tag="mm")
                        nc.tensor.matmul(
                            s_ps[:, :w],
                            kt_h[:, b * S + kb * 128:b * S + (kb + 1) * 128],
                            qt_h[:, b * S + j * 512 + q_off:
                                 b * S + (j + 1) * 512],
                            start=True, stop=True)
                        return s_ps

                    # scores issued 2 kb ahead so the PE stream never waits
                    # on the Exp drain of the previous block
                    sps = {0: issue_score(0)}
                    if nb > 1:
                        sps[1] = issue_score(1)
                    for kb in range(nb):
                        rr = kb - 4 * j
                        q_off = max(rr, 0) * 128
                        w = 512 - q_off
                        if kb + 2 < nb:
                            sps[kb + 2] = issue_score(kb + 2)
                        s_ps = sps.pop(kb)
                        ex = expp.tile([128, 512], BF16, tag="exp")
                        nc.scalar.activation(ex[:, :w], s_ps[:, :w],
                                             mybir.ActivationFunctionType.Exp,
                                             bias=zeros_b, scale=SCALE)
                        if rr >= 0:
                            nc.vector.tensor_mul(ex[:, 0:128], ex[:, 0:128],
                                                 mask_t)
                        nc.tensor.matmul(
                            pv_ps[:, q_off:512],
                            v_tiles[b * 16 + kb][:, h * 128:(h + 1) * 128],
                            ex[:, :w],
                            start=(kb == 0), stop=(kb == nb - 1))
                        nc.tensor.matmul(
                            den_ps[0:1, q_off:512], ones_col_b, ex[:, :w],
                            start=(kb == 0), stop=(kb == nb - 1))
                    inv_d = sqp.tile([1, 512], BF16, tag="invd", bufs=2)
                    with nc.allow_low_precision(
                            reason="softmax denom bcast in bf16"):
                        nc.vector.reciprocal(inv_d, den_ps)
                    bc = opp.tile([128, 512], F32, tag="op")
                    nc.tensor.matmul(bc, ones_row_b, inv_d,
                                     start=True, stop=True)
                    bc_sb = sqp.tile([128, 512], F32, tag="bcsb",
                                     name="bc_sb", bufs=1)
                    nc.scalar.copy(bc_sb, bc)
                    at = attnp.tile([128, 512], BF16, tag=f"at{h}",
                                    name=f"at{h}")
                    nc.vector.tensor_mul(at, pv_ps, bc_sb)
                    attn_j.append(at)

                # out projection for this (batch, token tile), then RS it
                for tbl in range(4):
                    ws = wop.tile([128, HID], BF16, tag="wo")
                    for cch in range(4):
                        po = opp.tile([128, 512], F32, tag="op")
                        for h in range(NHL):
                            nc.tensor.matmul(
                                po, attn_j[h][:, tbl * 128:(tbl + 1) * 128],
                                w_out_tiles[h][:, cch * 512:(cch + 1) * 512],
                                start=(h == 0), stop=(h == NHL - 1))
                        dst = ws[:, cch * 512:(cch + 1) * 512]
                        if cch % 2 == 0:
                            nc.vector.tensor_copy(dst, po)
                        else:
                            nc.scalar.copy(dst, po)
                    nc.sync.dma_start(
                        out=op_buf[p, tbl * 128:(tbl + 1) * 128, :], in_=ws)
                nc.gpsimd.collective_compute(
                    "ReduceScatter", mybir.AluOpType.add,
                    replica_groups=GROUPS,
                    ins=[op_buf[p]], outs=[rs_out[p]])

        # final copies rs_out -> out deferred to the end: issuing them
        # per-tile would park a DMA behind the in-flight RS on the FIFO DMA
        # queue and convoy the next tile's op_buf writes behind it
        for p in range(NTT):
            nc.sync.dma_start(out=out[p * 64:(p + 1) * 64, :], in_=rs_out[p])

    nc.finalize()
    return nc


def make_in_maps(x, w_in, w_out, q_norm_w, k_norm_w):
    x = np.asarray(x, np.float32)
    w_in = np.asarray(w_in, np.float32)
    w_out = np.asarray(w_out, np.float32)
    q_norm_w = np.asarray(q_norm_w, np.float32)
    k_norm_w = np.asarray(k_norm_w, np.float32)

    # [2048 hid, 4096 tok] batch-major tokens
    xT_full = np.concatenate([x[0].T, x[1].T], axis=1).astype(NPBF16)

    half = HD // 2
    inv_freq = 1.0 / (ROPE_BASE ** (np.arange(half, dtype=np.float32) / half))
    f2 = np.concatenate([inv_freq, inv_freq])            # [128]
    hi = np.arange(32, dtype=np.float32) * 64.0
    lo = np.arange(64, dtype=np.float32)
    angA = f2[:, None] * hi[None, :]                     # [128, 32]
    angB = f2[:, None] * lo[None, :]                     # [128, 64]
    ctab = np.concatenate([
        np.cos(angA), np.sin(angA),
        np.cos(angB), np.sin(angB),
    ], axis=1)                                           # [128, 192]
    maskT = (np.arange(128)[:, None] <= np.arange(128)[None, :])
    # lhsT of the rotate-half permutation: rot(x) = [-x2; x1]
    rotT = np.zeros((128, 128), np.float32)
    rotT[0:64, 64:128] = np.eye(64)
    rotT[64:128, 0:64] = -np.eye(64)

    in_maps = []
    for c in range(NCORES):
        rows = np.concatenate([
            w_in[c * DL:(c + 1) * DL],
            w_in[NH * HD + c * DL:NH * HD + (c + 1) * DL],
            w_in[2 * NH * HD + c * DL:2 * NH * HD + (c + 1) * DL],
        ], axis=0)                                  # [768, HID]
        aux = np.concatenate([
            q_norm_w[c * DL:(c + 1) * DL].reshape(NHL, 128).T,
            k_norm_w[c * DL:(c + 1) * DL].reshape(NHL, 128).T,
            ctab, maskT, rotT,
        ], axis=1).astype(np.float32)               # [128, 452]
        w_inT_c = np.ascontiguousarray(rows.T).astype(NPBF16)
        w_outT_c = np.ascontiguousarray(
            w_out[:, c * DL:(c + 1) * DL].T).astype(NPBF16)
        blob = np.concatenate([
            xT_full,
            w_inT_c.reshape(-1, TOK),
            w_outT_c.reshape(-1, TOK),
        ], axis=0)
        in_maps.append({
            "blob": np.ascontiguousarray(blob),
            "aux": np.ascontiguousarray(aux),
        })
    return in_maps


def assemble(results):
    """results[c] is [512, HID] bf16: rows p*64..(p+1)*64 are this core's rank
    slice of piece p = 4*b+j (batch b, tokens j*512 + c*64 ..)."""
    outp = np.empty((B, S, HID), np.float32)
    for c in range(NCORES):
        r = np.asarray(results[c], dtype=np.float32)
        for p in range(NTT):
            b, j = p // NT, p % NT
            t0 = j * 512 + c * 64
            outp[b, t0:t0 + 64, :] = r[p * 64:(p + 1) * 64, :]
    return outp


def kernel(x, w_in, w_out, q_norm_w, k_norm_w, trace=False):
    global LAST_EXEC_NS, _CACHED_NC
    if _CACHED_NC is None:
        _CACHED_NC = build_nc()
    nc = _CACHED_NC
    in_maps = make_in_maps(x, w_in, w_out, q_norm_w, k_norm_w)
    res = run_bass_kernel_spmd(nc, in_maps, list(range(NCORES)), trace=trace)
    LAST_EXEC_NS = res.exec_time_ns
    return assemble([res.results[c]["out"] for c in range(NCORES)])
